# revision 17
# baseline (speedup 1.0000x reference)
"""CrossNetV2 soft-MoE kernel for 8 axon-tunneled TRN2 NeuronCores.

Problem (hardcoded shapes): B=16384, D=1024, R=64, E=4, L=3.
    for l in range(L):
        h         = relu(einsum('bd,edr->ber', x, U[l]))          # [B,E,R]
        expert    = einsum('ber,erd->bed', h, V[l])               # [B,E,D]
        gate      = softmax(x @ gW[l] + gb[l])                    # [B,E]
        mixed     = einsum('be,bed->bd', gate, expert)            # [B,D]
        x         = x0 * mixed + bias[l] + x

Strategy:
  - Data parallel: batch split 8 ways (2048 rows/core); params replicated.
  - Feature-on-partitions layout throughout; host transposes x0 to [D, B_core].
  - Running-sum reformulation: x_l = x0 * M_l + B_l with
        M_l = 1 + sum_{j<=l} mixed_j,   B_l = sum_{j<=l} bias_j  (host const)
    so the per-layer epilogue is only:  M += mixed (DVE);  xb = bf16(M * x0)
    (GpSimd, chunk-paired). The affine shift B_{l-1} is folded into the next
    layer's matmuls via per-partition bias APs on the relu/exp activations
    (B@U on relu, B@gW + gb on exp) at zero cost. On the last layer M is
    preloaded into the PSUM bank and the V-matmuls accumulate on top
    (start=False), so the final add is free; y = M_f*x0 + B_2 via DVE + ACT.
  - Gate folded into h before the V contraction (mixed = (gate_bc*relu(h)) @ V);
    softmax denominator via ones-matmul, reciprocal_approx_fast on DVE,
    broadcasts back to 4/128 partitions via tiny PE matmuls.
  - Matmuls in bf16 (PSUM accumulates f32); M kept in f32 (precision anchor;
    bf16 rounding enters per layer but never compounds across layers).
  - Software pipelining: layer-major over the 4 batch tiles with ~1.5-phase
    lookahead; every PE op of the softmax chain for item i is emitted behind
    GEMM work of item i+1, so its cross-engine dependency has resolved when
    the PE reaches it. UNROLL=8 invocations are chained into one continuous
    pipeline per For_i iteration (the all-engine loop barrier costs ~16-25us,
    paid once per 8 invocations).
"""

import os
import sys

sys.path.insert(0, "/opt/trn_rl_repo")

import numpy as np

B, D, R, E, L = 16384, 1024, 64, 4, 3
N_CORES = 8
BC = B // N_CORES          # rows per core
BT = 512                   # batch-tile (free dim / PSUM bank width)
NBT = BC // BT             # b-tiles per core
ER = E * R                 # 256
DC = D // 128              # 8 d-chunks
EC = ER // 128             # 2 er-chunks

MM_DTYPE = os.environ.get("KMM_DTYPE", "bf16")   # "bf16" | "f32r"
XB_GPS = int(os.environ.get("KXB_GPS", "8"))     # d-chunks of xb-mul on GpSimd


def build_body(nc, tc, reps=1):
    import concourse.mybir as mybir

    f32 = mybir.dt.float32
    f32r = mybir.dt.float32r
    wdt = mybir.dt.bfloat16 if MM_DTYPE == "bf16" else f32r
    AF = mybir.ActivationFunctionType

    x0T = nc.dram_tensor("x0T", [D, BC], wdt, kind="ExternalInput")
    U_all = nc.dram_tensor("U_all", [L, D, ER], wdt, kind="ExternalInput")
    V_all = nc.dram_tensor("V_all", [L, ER, D], wdt, kind="ExternalInput")
    gW_all = nc.dram_tensor("gW_all", [L, D, E], wdt, kind="ExternalInput")
    sel_h = nc.dram_tensor("sel_h", [E, EC, 128], wdt, kind="ExternalInput")
    ones_h = nc.dram_tensor("ones_h", [4, 4], wdt, kind="ExternalInput")
    gbB_h = nc.dram_tensor("gbB_h", [E, L], f32, kind="ExternalInput")
    BU_h = nc.dram_tensor("BU_h", [128, L, EC], f32, kind="ExternalInput")
    B2_h = nc.dram_tensor("B2_h", [128, DC], f32, kind="ExternalInput")
    yT = nc.dram_tensor("yT", [D, BC], f32, kind="ExternalOutput")

    from contextlib import ExitStack

    ctx = ExitStack()
    const = ctx.enter_context(tc.tile_pool(name="const", bufs=1))
    x0p = ctx.enter_context(tc.tile_pool(name="x0p", bufs=NBT))
    Mp = ctx.enter_context(tc.tile_pool(name="Mp", bufs=NBT))
    xbp = ctx.enter_context(tc.tile_pool(name="xbp", bufs=NBT))
    hrp = ctx.enter_context(tc.tile_pool(name="hrp", bufs=4))
    hsp = ctx.enter_context(tc.tile_pool(name="hsp", bufs=3))
    smp = ctx.enter_context(tc.tile_pool(name="smp", bufs=3))
    gnp = ctx.enter_context(tc.tile_pool(name="gnp", bufs=2))
    tp = ctx.enter_context(tc.tile_pool(name="tp", bufs=3))
    ps_lg = ctx.enter_context(tc.tile_pool(name="ps_lg", bufs=3, space="PSUM"))
    ps_h = ctx.enter_context(tc.tile_pool(name="ps_h", bufs=2, space="PSUM"))
    ps_mx = ctx.enter_context(tc.tile_pool(name="ps_mx", bufs=3, space="PSUM"))

    # ---- constants (loaded once, outside the reps loop) ----
    Ur = const.tile([128, L, DC, ER], wdt)
    Vr = const.tile([128, L, EC, D], wdt)
    gWr = const.tile([128, L, DC, E], wdt)
    selr = const.tile([E, EC, 128], wdt)
    ones44 = const.tile([4, 4], wdt)
    gbBt = const.tile([E, L], f32)
    BUr = const.tile([128, L, EC], f32)
    B2t = const.tile([128, DC], f32)

    nc.sync.dma_start(out=Ur, in_=U_all.rearrange("l (c p) er -> p l c er", p=128))
    nc.sync.dma_start(out=Vr, in_=V_all.rearrange("l (k p) d -> p l k d", p=128))
    nc.sync.dma_start(out=gWr, in_=gW_all.rearrange("l (c p) e -> p l c e", p=128))
    nc.sync.dma_start(out=selr, in_=sel_h[:, :, :])
    nc.sync.dma_start(out=ones44, in_=ones_h[:, :])
    nc.sync.dma_start(out=gbBt, in_=gbB_h[:, :])
    nc.sync.dma_start(out=BUr, in_=BU_h[:, :, :])
    nc.sync.dma_start(out=B2t, in_=B2_h[:, :])

    # persistent per-tile state
    x0_t = [x0p.tile([128, DC, BT], wdt, tag="x0", name=f"x0_{i}") for i in range(NBT)]
    M_t = [Mp.tile([128, DC, BT], f32, tag="M", name=f"M_{i}") for i in range(NBT)]
    xb_t = [xbp.tile([128, DC, BT], wdt, tag="xb", name=f"xb_{i}") for i in range(NBT)]

    state = {}

    def ph_logits(key):
        r, l, t = key
        b0 = t * BT
        if l == 0:
            nc.sync.dma_start(
                out=x0_t[t],
                in_=x0T[:, b0 : b0 + BT].rearrange("(c p) b -> p c b", p=128),
            )
        xin = x0_t[t] if l == 0 else xb_t[t]
        logits = ps_lg.tile([E, BT], f32, tag="lg")
        for c in range(DC):
            nc.tensor.matmul(
                logits,
                gWr[:, l, c, :],
                xin[:, c, :],
                start=(c == 0),
                stop=(c == DC - 1),
            )
        explog = smp.tile([E, BT], wdt, tag="explog")
        nc.scalar.activation(
            out=explog, in_=logits, func=AF.Exp, bias=gbBt[:, l : l + 1], scale=1.0
        )
        state[key] = {"explog": explog, "hrs": [None, None]}

    def ph_h(key, m):
        r, l, t = key
        xin = x0_t[t] if l == 0 else xb_t[t]
        hm = ps_h.tile([128, BT], f32, tag="hp")
        for c in range(DC):
            nc.tensor.matmul(
                hm,
                Ur[:, l, c, m * 128 : (m + 1) * 128],
                xin[:, c, :],
                start=(c == 0),
                stop=(c == DC - 1),
            )
        hr = hrp.tile([128, BT], wdt, tag="hr")
        nc.scalar.activation(
            out=hr, in_=hm, func=AF.Relu, bias=BUr[:, l, m : m + 1], scale=1.0
        )
        state[key]["hrs"][m] = hr

    def ph_sum(key):
        r, l, t = key
        st = state[key]
        s = ps_lg.tile([1, BT], f32, tag="lg")
        nc.tensor.matmul(s, ones44[:, 0:1], st["explog"], start=True, stop=True)
        recip = smp.tile([1, BT], f32, tag="recip")
        nc.vector.reciprocal_approx_fast(out=recip, in_=s)
        recip_b = smp.tile([1, BT], wdt, tag="recipb")
        nc.scalar.copy(out=recip_b, in_=recip)
        st["recip_b"] = recip_b

    def ph_r4(key):
        st = state[key]
        r4 = ps_lg.tile([E, BT], f32, tag="lg")
        nc.tensor.matmul(r4, ones44[0:1, :], st["recip_b"], start=True, stop=True)
        gate_n = gnp.tile([E, BT], wdt, tag="gate_n")
        with nc.allow_low_precision(reason="gate in matmul dtype"):
            nc.vector.tensor_mul(out=gate_n, in0=st["explog"], in1=r4)
        st["gate_n"] = gate_n

    def ph_gbc(key):
        r, l, t = key
        st = state[key]
        hs = hsp.tile([128, EC, BT], wdt, tag="hs")
        for m in range(EC):
            gbc = ps_mx.tile([128, BT], f32, tag="mx")
            nc.tensor.matmul(gbc, selr[:, m, :], st["gate_n"], start=True, stop=True)
            with nc.allow_low_precision(reason="hs in matmul dtype"):
                nc.vector.tensor_mul(out=hs[:, m, :], in0=st["hrs"][m], in1=gbc)
        st["hs"] = hs
        if l == L - 1:
            # pre-allocate + preload the first mixed bank one phase early so
            # the PE's first V-matmul doesn't wait on the ACT copy.
            mx0 = ps_mx.tile([128, BT], f32, tag="mx")
            nc.scalar.copy(out=mx0, in_=M_t[t][:, 0, :])
            st["mx0"] = mx0

    def ph_mix(key):
        r, l, t = key
        b0 = t * BT
        st = state.pop(key)
        hs = st["hs"]
        last = l == L - 1
        for d in range(DC):
            if last and d == 0:
                mx = st["mx0"]
            else:
                mx = ps_mx.tile([128, BT], f32, tag="mx")
            if last and d > 0:
                # preload M into the PSUM bank; the V-matmuls accumulate on
                # top, yielding M_final = M + mixed with no extra DVE add.
                nc.scalar.copy(out=mx, in_=M_t[t][:, d, :])
            for k in range(EC):
                nc.tensor.matmul(
                    mx,
                    Vr[:, l, k, d * 128 : (d + 1) * 128],
                    hs[:, k, :],
                    start=(k == 0 and not last),
                    stop=(k == EC - 1),
                    skip_group_check=last,
                )
            if last:
                td = tp.tile([128, BT], f32, tag="t")
                nc.vector.tensor_mul(out=td, in0=mx, in1=x0_t[t][:, d, :])
                nc.scalar.activation(
                    out=td, in_=td, func=AF.Identity, bias=B2t[:, d : d + 1], scale=1.0
                )
                nc.sync.dma_start(
                    out=yT[:, b0 : b0 + BT].rearrange("(c p) b -> p c b", p=128)[
                        :, d, :
                    ],
                    in_=td,
                )
            else:
                if l == 0:
                    # M = mixed + 1  (f32 anchor)
                    nc.scalar.activation(
                        out=M_t[t][:, d, :], in_=mx, func=AF.Identity,
                        bias=1.0, scale=1.0,
                    )
                else:
                    nc.vector.tensor_add(
                        out=M_t[t][:, d, :], in0=M_t[t][:, d, :], in1=mx
                    )
                if d % 2 == 1:
                    # one fused [128, 2, BT] mul per d-pair (amortizes the
                    # per-op fixed cost on the engine)
                    eng = nc.gpsimd if d < XB_GPS else nc.vector
                    with nc.allow_low_precision(reason="xb in matmul dtype"):
                        eng.tensor_mul(
                            out=xb_t[t][:, d - 1 : d + 1, :],
                            in0=M_t[t][:, d - 1 : d + 1, :],
                            in1=x0_t[t][:, d - 1 : d + 1, :],
                        )

    def pipeline(unroll):
        # Software pipeline with ~1.5-step lookahead: every PE op of the
        # softmax chain for item i is emitted behind GEMM work of item i+1,
        # so its cross-engine dependency (DVE recip / gate_n, ACT exp/cast)
        # has already resolved when the PE reaches it. `unroll` reps are
        # chained into ONE pipeline so the For_i barrier bubble is paid only
        # once per `unroll` reps.
        items = [
            (r, l, t) for r in range(unroll) for l in range(L) for t in range(NBT)
        ]
        n = len(items)
        ph_logits(items[0])
        ph_h(items[0], 0)
        ph_h(items[0], 1)
        ph_sum(items[0])
        for i in range(n):
            if i + 1 < n:
                ph_logits(items[i + 1])
                ph_r4(items[i])
                ph_h(items[i + 1], 0)
                ph_gbc(items[i])
                ph_h(items[i + 1], 1)
                ph_sum(items[i + 1])
                ph_mix(items[i])
            else:
                ph_r4(items[i])
                ph_gbc(items[i])
                ph_mix(items[i])

    UNROLL = 8
    if reps == 1:
        pipeline(1)
    elif reps % UNROLL == 0:
        with tc.For_i(0, reps // UNROLL, 1):
            pipeline(UNROLL)
    else:
        with tc.For_i(0, reps, 1):
            pipeline(1)

    ctx.close()


def prep_inputs(x0, U, V, gW, gb, bias):
    """Host-side shard + transpose + param reshape. Returns list of per-core
    input dicts."""
    import ml_dtypes

    wnp = ml_dtypes.bfloat16 if MM_DTYPE == "bf16" else np.float32

    x0 = np.ascontiguousarray(np.asarray(x0, dtype=np.float32))
    U = np.asarray(U, dtype=np.float32)
    V = np.asarray(V, dtype=np.float32)
    gW = np.ascontiguousarray(np.asarray(gW, dtype=np.float32))
    gb = np.asarray(gb, dtype=np.float32)
    bias = np.asarray(bias, dtype=np.float32)

    U_all = np.ascontiguousarray(U.transpose(0, 2, 1, 3).reshape(L, D, ER))
    V_all = np.ascontiguousarray(V.reshape(L, ER, D))

    # B_l = sum_{j<=l} bias_j ; affine shifts folded into next-layer biases
    Bcum = np.cumsum(bias, axis=0)                     # [L, D], B_l
    Bprev = np.concatenate([np.zeros((1, D), np.float32), Bcum[:-1]], 0)  # B_{l-1}
    # exp bias: gb_l + B_{l-1} @ gW_l   -> [E, L]
    gbB = np.stack([gb[l] + Bprev[l] @ gW[l] for l in range(L)], axis=1)
    gbB = np.ascontiguousarray(gbB.astype(np.float32))
    # relu bias: B_{l-1} @ U_l  -> [L, ER] -> [128, L, EC]
    BU = np.stack([Bprev[l] @ U_all[l] for l in range(L)], axis=0)  # [L, ER]
    BU_t = np.ascontiguousarray(
        BU.reshape(L, EC, 128).transpose(2, 0, 1).astype(np.float32)
    )
    # final-layer bias: B_{L-1} [D] -> [128, DC]  (d = c*128 + p)
    B2_t = np.ascontiguousarray(Bcum[L - 1].reshape(DC, 128).T.astype(np.float32))

    sel = np.zeros((E, EC, 128), np.float32)
    for m in range(EC):
        for p in range(128):
            sel[2 * m + p // 64, m, p] = 1.0
    ones44 = np.ones((4, 4), np.float32)

    shared = dict(
        U_all=U_all.astype(wnp),
        V_all=V_all.astype(wnp),
        gW_all=gW.astype(wnp),
        sel_h=sel.astype(wnp),
        ones_h=ones44.astype(wnp),
        gbB_h=gbB,
        BU_h=BU_t,
        B2_h=B2_t,
    )
    in_maps = []
    for c in range(N_CORES):
        shard = x0[c * BC : (c + 1) * BC, :]                           # [BC, D]
        x0T = np.ascontiguousarray(shard.T).astype(wnp)                # [D, BC]
        m = dict(shared)
        m["x0T"] = x0T
        in_maps.append(m)
    return in_maps


_compiled = None


def _get_compiled():
    global _compiled
    if _compiled is None:
        import jax
        import numpy as _np
        from jax.sharding import Mesh, PartitionSpec, NamedSharding
        from jax.experimental.shard_map import shard_map
        import concourse.bacc as bacc
        import concourse.mybir as mybir
        from concourse import bass2jax
        from concourse.tile import TileContext

        nc = bacc.Bacc(
            "TRN2", target_bir_lowering=False, debug=False, num_devices=N_CORES
        )
        with TileContext(nc) as tc:
            build_body(nc, tc, reps=1)
        nc.compile()

        in_names, out_names, out_avals = [], [], []
        zero_shapes = []
        for alloc in nc.m.functions[0].allocations:
            if not isinstance(alloc, mybir.MemoryLocationSet):
                continue
            name = alloc.memorylocations[0].name
            if alloc.kind == "ExternalInput":
                in_names.append(name)
            elif alloc.kind == "ExternalOutput":
                out_names.append(name)
                shape = tuple(alloc.tensor_shape)
                dtype = mybir.dt.np(alloc.dtype)
                out_avals.append(jax.core.ShapedArray(shape, dtype))
                zero_shapes.append((shape, dtype))

        def _bass_body(*args):
            outs = bass2jax._bass_exec_p.bind(
                *args,
                out_avals=tuple(out_avals),
                in_names=tuple(in_names + out_names),
                out_names=tuple(out_names),
                lowering_input_output_aliases=(),
                sim_require_finite=True,
                sim_require_nnan=True,
                nc=nc,
            )
            return tuple(outs)

        devices = jax.devices()[:N_CORES]
        mesh = Mesh(_np.asarray(devices), ("core",))
        n_params, n_outs = len(in_names), len(out_names)
        fn = jax.jit(
            shard_map(
                _bass_body,
                mesh=mesh,
                in_specs=(PartitionSpec("core"),) * (n_params + n_outs),
                out_specs=(PartitionSpec("core"),) * n_outs,
                check_rep=False,
            ),
            keep_unused=True,
        )
        sharding = NamedSharding(mesh, PartitionSpec("core"))
        _compiled = (fn, in_names, out_names, out_avals, zero_shapes, sharding)
    return _compiled


def run_device(in_maps):
    """Run the compiled NEFF on 8 cores; returns list of per-core out dicts."""
    import jax

    fn, in_names, out_names, out_avals, zero_shapes, sharding = _get_compiled()
    concat = []
    for n in in_names:
        if n == "partition_id":
            concat.append(
                np.arange(N_CORES, dtype=np.uint32).reshape(N_CORES, 1)
            )
        else:
            concat.append(
                np.concatenate([np.asarray(m[n]) for m in in_maps], axis=0)
            )
    zeros = [np.zeros((N_CORES * s[0], *s[1:]), d) for (s, d) in zero_shapes]
    dev_args = [jax.device_put(a, sharding) for a in concat + zeros]
    outs = fn(*dev_args)
    jax.block_until_ready(outs)
    res = []
    for c in range(N_CORES):
        d = {}
        for i, name in enumerate(out_names):
            shape = out_avals[i].shape
            d[name] = np.asarray(outs[i]).reshape(N_CORES, *shape)[c]
        res.append(d)
    return res


def kernel(x0, U, V, gW, gb, bias):
    in_maps = prep_inputs(x0, U, V, gW, gb, bias)
    res = run_device(in_maps)
    out = np.empty((B, D), np.float32)
    for c in range(N_CORES):
        out[c * BC : (c + 1) * BC, :] = res[c]["yT"].T
    return out
